# revision 1
# baseline (speedup 1.0000x reference)
"""Trainium2 Bass kernel for nn_Autoregression (16-state AR whitening log-prob).

Math: reference computes log_prob[b,k,t] = -0.5*(C*log(2pi) + logdet(Sigma_k)
+ es_k(t)^T Sigma_k^{-1} es_k(t)) with es = causal_conv(x, W, b).  Since
Sigma^{-1} = L^{-T} L^{-1} and es is affine in x, fold L^{-1} into the conv:
W2 = L^{-1} W, b2 = L^{-1} b, then mahalanobis = sum_c conv(x; W2, b2)^2.

Device layout (per core, T sharded 8 ways with an 8-sample left halo):
conv as matmuls over 128-t chunks producing PSUM [128 t, 512 (8 states x 64
ch)] x 2 halves; contraction packed as (c_in x 2 time-shifts)=128 rows per
step, 4 steps + a 65-row step for the j=8 tap whose ones-row carries the
bias.  ACT squares PSUM -> bf16 SBUF; DVE does the per-state segmented
reduce [128, 8, 64] -> [128, 8]; a small PE transpose flips [128 t, 16 k]
-> [16 k, 128 t]; DVE applies -0.5 and the per-state constant; DMA out.
"""

import os

import numpy as np
import ml_dtypes

import concourse.bass as bass
import concourse.bacc as bacc_mod
import concourse.mybir as mybir
import concourse.tile as tile
from concourse.bass_utils import run_bass_kernel_spmd
from concourse.tile_rust import add_dep_helper

K = 16          # states
C = 64          # channels
T = 65536       # time
AR = 8          # ar order (kernel size AR+1)
NCORES = 8
TLOC = T // NCORES          # 8192 outputs per core
TC = 128                    # outputs per chunk (matmul M)
WAVE = 16                   # chunks per wave (input tile granularity)
WCOLS = TC * WAVE           # 2048 outputs per wave
NW = TLOC // WCOLS          # waves per core
KP = K // 2
NSTEP = 5                   # contraction steps: 4 full + 1 (j=8 + bias row)
NH = 2                      # psum halves (states 0-7, 8-15)

MM_DT = mybir.dt.bfloat16   # conv matmul dtype
SQ_DT = mybir.dt.bfloat16   # squares dtype

_MM_NP = mybir.dt.np(MM_DT)

_CACHE: dict = {}


def _build_program():
    nc = bacc_mod.Bacc()
    f32 = mybir.dt.float32

    # xin rows 0-63: x slice (with halo); rows 64-127: same shifted left by 1
    # (host-duplicated so each wave's xd tile loads with a single DMA)
    xin = nc.declare_dram_parameter("xin", [128, TLOC + AR], MM_DT, isOutput=False)
    # weights as the matmul moving operand: [contraction, step, (half, state, ch)]
    wts = nc.declare_dram_parameter("wts", [128, NSTEP, 1024], MM_DT, isOutput=False)
    ident = nc.declare_dram_parameter("ident", [128, 128], mybir.dt.float32r, isOutput=False)
    biasc = nc.declare_dram_parameter("biasc", [K, 1], f32, isOutput=False)
    onesd = nc.declare_dram_parameter("onesd", [1, WCOLS], MM_DT, isOutput=False)
    out = nc.declare_dram_parameter("out", [K, TLOC], f32, isOutput=True)

    XDW = WCOLS + AR

    with tile.TileContext(nc) as tc:
        with (
            tc.tile_pool(name="singles", bufs=1) as singles,
            # one slot per wave: input DMAs never wait (no slot WAR/WAW)
            tc.tile_pool(name="xpool", bufs=NW) as xpool,
            tc.tile_pool(name="sqpool", bufs=12) as sqpool,
            tc.tile_pool(name="mpool", bufs=6) as mpool,
            tc.tile_pool(name="conv_ps", bufs=5, space="PSUM") as conv_ps,
            tc.tile_pool(name="mt_ps", bufs=2, space="PSUM") as mt_ps,
            tc.tile_pool(name="obs_ps", bufs=1, space="PSUM") as obs_ps,
        ):
            # Matmuls must never be the first PE instruction to observe more
            # than one producer semaphore (1-wait ISA slots; bacc's event-sem
            # legalization costs sequencer time).  pe_observe() emits a tiny
            # 2x2 "reader" matmul whose operands come from a single
            # producer's tile; ordering edges pin readers ahead of the next
            # real matmul.
            scratch = obs_ps.tile([2, 128], f32)
            scratch2 = singles.tile([2, 128], SQ_DT)
            nc.vector.memset(scratch2, 0.0)
            pending = []
            obs_after = [None]

            def pe_observe(col):
                i = nc.tensor.matmul(
                    scratch[0:2, 0:2], col, col, start=True, stop=True
                )
                if obs_after[0] is not None:
                    # not earlier than late in the previous wave, or the PE
                    # FIFO head-of-line blocks on a DMA that hasn't landed
                    add_dep_helper(i.ins, obs_after[0].ins, sync=False)
                pending.append(i)

            def _flush(i):
                while pending:
                    add_dep_helper(i.ins, pending.pop().ins, sync=False)
                return i

            def pe_matmul(*args, **kw):
                return _flush(nc.tensor.matmul(*args, **kw))

            # dep-free warmup matmuls: keep the PE busy through the initial
            # input DMAs so HAM un-throttles before real work (N=128 streams
            # so the activity monitor sees a busy array)
            for _ in range(35):
                nc.tensor.matmul(
                    scratch[0:2, 0:128],
                    scratch2[0:2, 0:2],
                    scratch2[0:2, 0:128],
                    start=True,
                    stop=True,
                )

            # DMA issue plan: sync HWDGE ring carries the critical path
            # (first xd half, per-step weights, second xd half);
            # prefetchables (identity, bias, xe, waves 1-3) go on the scalar
            # engine's separate ring.
            w_sb = singles.tile([128, NSTEP, 1024], MM_DT)
            ident_sb = singles.tile([128, 128], mybir.dt.float32r)
            bias_sb = singles.tile([K, 1], f32)
            out_sb = singles.tile([K, TLOC], f32)
            xds, xes = [], []
            sc_dmas = []
            sc_dmas.append(nc.scalar.dma_start(out=ident_sb, in_=ident[:, :]))
            sc_dmas.append(nc.scalar.dma_start(out=bias_sb, in_=biasc[:, :]))
            for w in range(NW):
                base = w * WCOLS
                # xd: rows 0-63 = xin shifts (j even), rows 64-127 = xin
                # shifted one further (j odd).  xe: rows 0-63 = xin shift 8,
                # row 64 = ones (bias row).
                xd = xpool.tile([128, XDW], MM_DT, name="xd")
                xe = xpool.tile([C + 1, WCOLS], MM_DT, name="xe")
                if w == 0:
                    nc.sync.dma_start(out=xd[:, 0:1036], in_=xin[:, 0:1036])
                    for s in range(NSTEP):
                        nc.sync.dma_start(
                            out=w_sb[:, s, :], in_=wts[:, s, :]
                        )
                    nc.sync.dma_start(out=xd[:, 1036:XDW], in_=xin[:, 1036:XDW])
                    sc_dmas.append(
                        nc.scalar.dma_start(
                            out=xe[0:C, :],
                            in_=xin[0:C, base + AR : base + AR + WCOLS],
                        )
                    )
                    sc_dmas.append(
                        nc.scalar.dma_start(out=xe[C : C + 1, :], in_=onesd[:, :])
                    )
                elif w == 1:
                    sc_dmas.append(
                        nc.scalar.dma_start(out=xd, in_=xin[:, base : base + XDW])
                    )
                    sc_dmas.append(
                        nc.scalar.dma_start(
                            out=xe[0:C, :],
                            in_=xin[0:C, base + AR : base + AR + WCOLS],
                        )
                    )
                    sc_dmas.append(
                        nc.scalar.dma_start(out=xe[C : C + 1, :], in_=onesd[:, :])
                    )
                xds.append(xd)
                xes.append(xe)

            def load_wave_inputs(w):
                # waves 2-3 load lazily (two waves ahead) so the prefetch
                # doesn't flood the DMA fabric while wave 0 computes
                base = w * WCOLS
                nc.scalar.dma_start(out=xds[w], in_=xin[:, base : base + XDW])
                nc.scalar.dma_start(
                    out=xes[w][0:C, :],
                    in_=xin[0:C, base + AR : base + AR + WCOLS],
                )
                nc.scalar.dma_start(out=xes[w][C : C + 1, :], in_=onesd[:, :])
            # DVE observer for the bias DMA (TS struct fits one wait)
            dve_scratch = singles.tile([K, 1], f32)
            nc.vector.tensor_copy(dve_scratch, bias_sb)

            first_sq = [True]

            def chunk_tail(w, off, psh):
                base = w * WCOLS
                m_sb = mpool.tile([128, K], mybir.dt.float32r, name="m_sb")
                for h in range(NH):
                    sq = sqpool.tile([128, 512], SQ_DT, name="sq", tag="sq")
                    sq_i = nc.scalar.activation(
                        sq, psh[h], mybir.ActivationFunctionType.Square
                    )
                    if first_sq[0]:
                        # the Act sequencer must issue every prefetch DMA
                        # before its first square, else a square that
                        # transitively gates one of those DMAs deadlocks
                        while sc_dmas:
                            add_dep_helper(sq_i.ins, sc_dmas.pop().ins, sync=False)
                        first_sq[0] = False
                    with nc.allow_low_precision(
                        reason="float32r shares float32 bits; r-mode only "
                        "affects the PE multiply path"
                    ):
                        nc.vector.tensor_reduce(
                            out=m_sb[:, 8 * h : 8 * h + 8],
                            in_=sq.rearrange("p (g c) -> p g c", g=8),
                            axis=mybir.AxisListType.X,
                            op=mybir.AluOpType.add,
                        )
                mt = mt_ps.tile([K, TC], mybir.dt.float32r, name="mt")
                _flush(nc.tensor.transpose(mt, m_sb, ident_sb))
                nc.vector.tensor_scalar(
                    out=out_sb[:, base + off : base + off + TC],
                    in0=mt[0:K, :],
                    scalar1=-0.5,
                    scalar2=bias_sb,
                    op0=mybir.AluOpType.mult,
                    op1=mybir.AluOpType.add,
                )

            def conv_lhsT(xd, xe, off, s):
                if s < 4:
                    return xd[:, off + 2 * s : off + 2 * s + TC]
                return xe[0 : C + 1, off : off + TC]

            def conv_rhs(s, h):
                if s < 4:
                    return w_sb[:, s, 512 * h : 512 * h + 512]
                return w_sb[0 : C + 1, s, 512 * h : 512 * h + 512]

            for w in range(NW):
                base = w * WCOLS
                xd = xds[w]
                xe = xes[w]
                if w + 2 < NW:
                    load_wave_inputs(w + 2)
                if w == 0:
                    # first four chunks pairwise s-major: the weight steps
                    # arrive one-by-one on the sync ring, so advance both
                    # chunks per step instead of stalling per chunk
                    for pair in ((0, 1), (2, 3)):
                        pshp = {
                            (c, h): conv_ps.tile(
                                [128, 512], f32, name=f"ps{c}{h}", tag="ps"
                            )
                            for c in pair
                            for h in range(NH)
                        }
                        for s in range(NSTEP):
                            if pair[0] == 0 and s == 0:
                                pe_observe(ident_sb[:, 0:2])
                                pe_observe(xd[:, 0:2])
                            if pair[0] == 0 and s == 4:
                                pe_observe(xe[0:C, 0:2])
                                pe_observe(xe[C : C + 1, 0:2])
                            for c in pair:
                                lhsT = conv_lhsT(xd, xe, c * TC, s)
                                for h in range(NH):
                                    pe_matmul(
                                        pshp[c, h],
                                        lhsT,
                                        conv_rhs(s, h),
                                        start=(s == 0),
                                        stop=(s == 4),
                                    )
                        for c in pair:
                            chunk_tail(w, c * TC, [pshp[c, h] for h in range(NH)])
                    start_tcl = 4
                else:
                    start_tcl = 0
                for tcl in range(start_tcl, WAVE):
                    off = tcl * TC
                    psh = [
                        conv_ps.tile([128, 512], f32, name=f"ps{h}", tag="ps")
                        for h in range(NH)
                    ]
                    for s in range(NSTEP):
                        if w > 0 and tcl == 0 and s == 0:
                            pe_observe(xd[:, 0:2])
                        if w > 0 and tcl == 0 and s == 4:
                            # lazily: s0-s3 must not stall on the xe loads
                            pe_observe(xe[0:C, 0:2])
                            pe_observe(xe[C : C + 1, 0:2])
                        lhsT = conv_lhsT(xd, xe, off, s)
                        for h in range(NH):
                            mm_i = pe_matmul(
                                psh[h],
                                lhsT,
                                conv_rhs(s, h),
                                start=(s == 0),
                                stop=(s == 4),
                            )
                    if tcl == WAVE - 2:
                        obs_after[0] = mm_i
                    chunk_tail(w, off, psh)
                if w < NW - 1:
                    nc.sync.dma_start(
                        out=out[:, base : base + WCOLS],
                        in_=out_sb[:, base : base + WCOLS],
                    )
                else:
                    # last wave: quarter DMAs so the final store is tiny
                    for q in range(4):
                        uq = base + q * (WCOLS // 4)
                        nc.sync.dma_start(
                            out=out[:, uq : uq + WCOLS // 4],
                            in_=out_sb[:, uq : uq + WCOLS // 4],
                        )
    nc.compile()
    return nc


def _prep_host(W, b, Sigma):
    """Fold L^{-1} into conv weights; pack moving-operand tiles, constants."""
    W64 = W.astype(np.float64)
    b64 = b.astype(np.float64)
    S64 = Sigma.astype(np.float64)
    L = np.linalg.cholesky(S64)
    Li = np.linalg.inv(L)                       # [K, C, C] lower-triangular inv
    logdet = 2.0 * np.sum(np.log(np.diagonal(L, axis1=1, axis2=2)), axis=1)
    W2 = np.einsum("kdc,kcij->kdij", Li, W64)   # [K, C(d), C(ci), 9]
    b2 = np.einsum("kdc,kc->kd", Li, b64)       # [K, C]

    # moving operand: w_np[r, s, 512*(k//8) + 64*(k%8) + d]
    #   s<4:  r = ci + 64*joff -> W2[k, d, ci, 2s+joff]
    #   s==4: r<64 -> W2[k, d, r, 8]; r==64 -> b2[k, d]; else 0
    w_np = np.zeros((128, NSTEP, 1024), np.float32)
    for s in range(4):
        # [ci + 64*joff, (k, d)]
        sub = W2[:, :, :, 2 * s : 2 * s + 2]        # [k, d, ci, joff]
        w_np[:, s, :] = np.transpose(sub, (3, 2, 0, 1)).reshape(128, 1024)
    w_np[0:C, 4, :] = np.transpose(W2[:, :, :, 8], (2, 0, 1)).reshape(C, 1024)
    w_np[C, 4, :] = b2.reshape(1024)

    const = C * np.log(2.0 * np.pi) + logdet
    bias_np = (-0.5 * const).astype(np.float32).reshape(K, 1)
    return w_np, bias_np


def _run(x, W, b, Sigma, trace=False):
    x = np.asarray(x, np.float32)
    W = np.asarray(W, np.float32)
    b = np.asarray(b, np.float32)
    Sigma = np.asarray(Sigma, np.float32)
    if "nc" not in _CACHE:
        _CACHE["nc"] = _build_program()
    nc = _CACHE["nc"]
    w_np, bias_np = _prep_host(W, b, Sigma)

    # left causal pad (AR) plus one right pad col so the shifted copy of the
    # last core's slice stays in bounds
    xpad = np.pad(np.asarray(x, np.float32)[0], ((0, 0), (AR, 1)))  # [C, T+9]
    in_maps = []
    for i in range(NCORES):
        lo = xpad[:, TLOC * i : TLOC * i + TLOC + AR]
        hi = xpad[:, TLOC * i + 1 : TLOC * i + TLOC + AR + 1]
        in_maps.append(
            {
                "xin": np.ascontiguousarray(
                    np.concatenate([lo, hi], axis=0).astype(_MM_NP)
                ),
                "wts": w_np.astype(_MM_NP),
                "ident": np.eye(128, dtype=np.float32),
                "biasc": bias_np,
                "onesd": np.ones((1, WCOLS), _MM_NP),
            }
        )
    res = run_bass_kernel_spmd(
        nc, in_maps, core_ids=list(range(NCORES)), trace=trace
    )
    outs = [res.results[i]["out"] for i in range(NCORES)]
    full = np.concatenate(outs, axis=1)[None]   # [1, K, T]
    return full.astype(np.float32), res


def kernel(x, W, b, Sigma):
    out, _ = _run(x, W, b, Sigma, trace=bool(int(os.environ.get("BASS_TRACE", "0"))))
    return out



# revision 4
# speedup vs baseline: 1.3242x; 1.3242x over previous
"""Trainium2 Bass kernel for nn_Autoregression — fp8 DoubleRow version.

Math: log_prob[b,k,t] = -0.5*(C*log(2pi) + logdet(Sigma_k)
+ ||L_k^{-1}(conv(x,W_k)+b_k)||^2).  Fold L^{-1} into the conv (W2, b2),
then mahal = sum_c es^2 with es = conv(x; W2, b2).

Device layout (per core, T sharded 8 ways, 8-sample left halo):
es computed as [kc, t] PSUM tiles (kc = 2 states x 64 ch per block, 8
blocks) so the channel reduction runs on the PE.  Conv contraction 577 =
(64ci x 9taps + bias) done in 3 fp8 DoubleRow matmuls per block per
512-t block (2x128-row k-tiles each, weights stationary, x moving with
a stride-2 overlapped AP).  Act squares groups 0-2 (PSUM -> fp8 SBUF);
group 3 is copied to bf16 SBUF by DVE and squared to fp8 by GpSimd
(DVE TensorTensor cannot read PSUM on both ports; walrus rejects it).
The tap-8+bias step uses a full 128-partition xed tile (row 64 = ones,
rows 65-127 = zeros w/ zero weights): 65-partition DoubleRow wedges the
device.  Its second k-tile points at a fixed zero dead-zone column
range (zero weights), keeping the AP a plain strided one.
4 DoubleRow mask matmuls reduce 64-channel groups -> mahal PSUM
[16, t]; DVE tensor_scalar applies -0.5*x+bias -> out SBUF f32;
sync-ring DMA out.  PE waits are kept single-semaphore via observer
matmuls (baseline trick).
"""

import math
import os

import numpy as np
import ml_dtypes

import concourse.bass as bass
import concourse.bacc as bacc_mod
import concourse.mybir as mybir
import concourse.tile as tile
from concourse.bass_utils import run_bass_kernel_spmd
from concourse.tile_rust import add_dep_helper
import bass_rust

K = 16
C = 64
T = 65536
AR = 8
NCORES = 8
TLOC = T // NCORES
TB = 512                    # t per block-iteration
NB = 8                      # kc blocks (2 states x 64ch each)
NG = 4                      # es groups per t-block (2 kc blocks each)
NS = 3                      # conv contraction steps (256+256+65)

FP8 = mybir.dt.float8e4
_FP8_NP = ml_dtypes.float8_e4m3

_CACHE: dict = {}


def _chunks(tloc):
    """xin/xe DMA chunks: [0, 520), then 4-t-block strides with halo."""
    ntb = tloc // TB
    bnds = [0, 1] + [1 + 4 * i for i in range(1, (ntb + 2) // 4)] + [ntb]
    bnds = sorted(set(b for b in bnds if b <= ntb))
    out = []
    for a, b in zip(bnds[:-1], bnds[1:]):
        lo = a * TB
        hi = (b - 1) * TB + TB + AR  # last col read: (b-1)*TB + 4s+2i+511 <= +519
        out.append((a, lo, min(hi, tloc + AR + 1)))
    return out


def _build_program(tloc=TLOC):
    nc = bacc_mod.Bacc()
    f32 = mybir.dt.float32
    ntb = tloc // TB

    xin = nc.declare_dram_parameter("xin", [128, tloc + AR + 1], FP8, isOutput=False)
    wts = nc.declare_dram_parameter("wts", [128, 6 * NB, 128], FP8, isOutput=False)
    maskd = nc.declare_dram_parameter("maskd", [128, 8, K], FP8, isOutput=False)
    biasd = nc.declare_dram_parameter("biasd", [K, 1], f32, isOutput=False)
    xed = nc.declare_dram_parameter("xed", [128, tloc + TB], FP8, isOutput=False)
    out = nc.declare_dram_parameter("out", [K, tloc], f32, isOutput=True)

    chunks = _chunks(tloc)

    with tile.TileContext(nc) as tc:
        with (
            tc.tile_pool(name="singles", bufs=1) as singles,
            tc.tile_pool(name="sqpool", bufs=2) as sqpool,
            tc.tile_pool(name="esbpool", bufs=2) as esbpool,
            tc.tile_pool(name="es_ps", bufs=3, space="PSUM") as es_ps,
            tc.tile_pool(name="m_ps", bufs=1, space="PSUM") as m_ps,
            tc.tile_pool(name="obs_ps", bufs=1, space="PSUM") as obs_ps,
        ):
            # --- observer machinery (single-sem matmul waits; see baseline) ---
            scratch = obs_ps.tile([2, 128], f32)
            scratch2 = singles.tile([2, 128], mybir.dt.bfloat16)
            nc.vector.memset(scratch2, 0.0)
            pending = []

            def pe_observe(col):
                i = nc.tensor.matmul(
                    scratch[0:2, 0:2], col, col, start=True, stop=True
                )
                pending.append(i)

            def _flush(i):
                while pending:
                    add_dep_helper(i.ins, pending.pop().ins, sync=False)
                return i

            # dep-free warmups: ramp PE/HAM while initial DMAs land
            for _ in range(35):
                nc.tensor.matmul(
                    scratch[0:2, 0:128],
                    scratch2[0:2, 0:2],
                    scratch2[0:2, 0:128],
                    start=True,
                    stop=True,
                )

            # --- SBUF tiles ---
            w_sb = singles.tile([128, 6 * NB, 128], FP8)
            mask_sb = singles.tile([128, 8, K], FP8)
            bias_sb = singles.tile([K, 1], f32)
            dummy_sb = singles.tile([K, 1], f32)
            xin_sb = singles.tile([128, tloc + AR + 1], FP8)
            xed_sb = singles.tile([128, tloc + TB], FP8)
            out_sb = singles.tile([K, tloc], f32)

            # --- input DMAs: all on the sync ring (keeps Act/DVE/Pool
            # queues free of DMA issue cost).  Critical-path order first.
            a0, lo0, hi0 = chunks[0]
            nc.sync.dma_start(out=bias_sb, in_=biasd[:, :])
            nc.sync.dma_start(out=w_sb[:, 0:6, :], in_=wts[:, 0:6, :])
            nc.sync.dma_start(out=xin_sb[:, lo0:hi0], in_=xin[:, lo0:hi0])
            nc.sync.dma_start(
                out=xed_sb[:, lo0 : min(hi0, tloc)], in_=xed[:, lo0 : min(hi0, tloc)]
            )
            nc.sync.dma_start(out=xed_sb[:, tloc:], in_=xed[:, tloc:])
            nc.sync.dma_start(out=mask_sb, in_=maskd[:, :, :])
            nc.sync.dma_start(out=w_sb[:, 6:48, :], in_=wts[:, 6:48, :])
            for a, lo, hi in chunks[1:]:
                nc.sync.dma_start(out=xin_sb[:, lo:hi], in_=xin[:, lo:hi])
                he = min(hi, tloc)
                nc.sync.dma_start(out=xed_sb[:, lo:he], in_=xed[:, lo:he])

            # DVE: touch bias first (absorbs its DMA sem on DVE's in-order
            # stream before the first affine tensor_scalar needs it)
            nc.vector.tensor_copy(dummy_sb, bias_sb)

            chunk_starts = {a: i for i, (a, lo, hi) in enumerate(chunks)}

            def conv_rhs(tb, s):
                """moving x AP for step s: [128(p), 2(ktile), TB] overlapped."""
                off = tb * TB
                if s < 2:
                    ap = xin_sb[:, off + 4 * s : off + 4 * s + TB].copy()
                    p = ap.ap[0]
                    ap.ap = bass_rust.VecI64Pair([[p[0], p[1]], [2, 2], [1, TB]])
                else:
                    # tile0 = tap8+bias cols; tile1 = fixed zero dead zone at
                    # [tloc, tloc+TB) (zero weights) via static stride
                    ap = xed_sb[:, off : off + TB].copy()
                    p = ap.ap[0]
                    ap.ap = bass_rust.VecI64Pair(
                        [[p[0], p[1]], [tloc - off, 2], [1, TB]]
                    )
                return ap

            def conv_lhsT(b, s):
                if s < 2:
                    return w_sb[:, 6 * b + 2 * s : 6 * b + 2 * s + 2, :]
                return w_sb[:, 6 * b + 4 : 6 * b + 6, :]

            DR = mybir.MatmulPerfMode.DoubleRow
            mlast = {}

            def emit_masks(tb):
                m = m_ps.tile([K, TB], f32, name="m", tag="m")
                sq = sqtiles[tb]
                for p in range(4):
                    i = nc.tensor.matmul(
                        m[:, :],
                        mask_sb[:, 2 * p : 2 * p + 2, :],
                        sq[:, 2 * p : 2 * p + 2, :],
                        start=(p == 0),
                        stop=(p == 3),
                        perf_mode=DR,
                    )
                    if p == 0:
                        _flush(i)
                mlast[tb] = m

            def emit_affine(tb):
                nc.vector.tensor_scalar(
                    out=out_sb[:, tb * TB : (tb + 1) * TB],
                    in0=mlast.pop(tb)[:, :],
                    scalar1=-0.5,
                    scalar2=bias_sb,
                    op0=mybir.AluOpType.mult,
                    op1=mybir.AluOpType.add,
                )

            sqtiles = {}
            for tb in range(ntb):
                sq = sqpool.tile([128, NB, TB], FP8, name="sq", tag="sq")
                sqtiles[tb] = sq
                for g in range(NG):
                    es = es_ps.tile([128, 2, TB], f32, name="es", tag="es")
                    if g == 0:
                        if tb in chunk_starts:
                            off = tb * TB
                            pe_observe(xin_sb[:, off : off + 2])
                            pe_observe(xed_sb[:, off : off + 2])
                        if tb == 0:
                            pe_observe(w_sb[:, 0, 0:2])
                            pe_observe(xed_sb[:, tloc : tloc + 2])
                    for h in range(2):
                        b = 2 * g + h
                        if tb == 0 and b == 1:
                            pe_observe(w_sb[:, 6, 0:2])
                        for s in range(NS):
                            i = nc.tensor.matmul(
                                es[:, h, :],
                                conv_lhsT(b, s),
                                conv_rhs(tb, s),
                                start=(s == 0),
                                stop=(s == 2),
                                perf_mode=DR,
                            )
                            if s == 0:
                                _flush(i)
                    with nc.allow_low_precision(
                        reason="squares quantized to fp8; validated host-side "
                        "(rel err 1.5e-2 vs 2e-2 budget)"
                    ):
                        if g < 3:
                            nc.scalar.activation(
                                sq[:, 2 * g : 2 * g + 2, :],
                                es[:, :, :],
                                mybir.ActivationFunctionType.Square,
                            )
                        else:
                            esb = esbpool.tile(
                                [128, 2, TB], mybir.dt.bfloat16, name="esb", tag="esb"
                            )
                            nc.vector.tensor_copy(esb, es[:, :, :])
                            nc.gpsimd.tensor_tensor(
                                sq[:, 6:8, :], esb, esb, mybir.AluOpType.mult
                            )
                    if g == 1 and tb > 0:
                        if tb == 1:
                            pe_observe(mask_sb[:, 0, 0:2])
                        else:
                            # absorb the DVE affine(tb-2) sem (m-tile WAR)
                            t2 = (tb - 2) * TB
                            pe_observe(out_sb[0:2, t2 : t2 + 2])
                        emit_masks(tb - 1)
                    if g == 2 and tb > 0:
                        emit_affine(tb - 1)
                    if g == 3 and tb > 0:
                        t0 = (tb - 1) * TB
                        nc.sync.dma_start(
                            out=out[:, t0 : t0 + TB], in_=out_sb[:, t0 : t0 + TB]
                        )
            t2 = (ntb - 2) * TB
            pe_observe(out_sb[0:2, t2 : t2 + 2])
            emit_masks(ntb - 1)
            emit_affine(ntb - 1)
            t0 = (ntb - 1) * TB
            nc.sync.dma_start(out=out[:, t0 : t0 + TB], in_=out_sb[:, t0 : t0 + TB])
    nc.compile()
    return nc


def _prep_host(W, b, Sigma):
    """Fold L^{-1} into conv weights; pack fp8 DoubleRow tiles + constants."""
    W64 = W.astype(np.float64)
    b64 = b.astype(np.float64)
    S64 = Sigma.astype(np.float64)
    L = np.linalg.cholesky(S64)
    Li = np.linalg.inv(L)
    logdet = 2.0 * np.sum(np.log(np.diagonal(L, axis1=1, axis2=2)), axis=1)
    W2 = np.einsum("kdc,kcij->kdij", Li, W64)   # [K, d, ci, 9]
    b2 = np.einsum("kdc,kc->kd", Li, b64)       # [K, d]

    W2q = W2.astype(np.float32).astype(_FP8_NP).astype(np.float32)
    b2q = b2.astype(np.float32).astype(_FP8_NP).astype(np.float32)

    # w_np[r, 6b+2s+i, m]: m = 64*(k-2b) + d
    #   s<2: = W2[2b + m//64, m%64, r%64, 4s+2i + r//64]
    #   s=2,i=0: r<64 -> W2[.., r, 8]; r==64 -> b2; else 0.  i=1: 0
    w_np = np.zeros((128, 48, 128), np.float32)
    # [k, d, ci, j] -> view [b, kin2, d, ci, j]
    Wb = W2q.reshape(NB, 2, C, C, 9)
    for b_ in range(NB):
        for s in range(2):
            for i in range(2):
                for par in range(2):
                    j = 4 * s + 2 * i + par
                    # rows par*64 + ci, cols kin2*64 + d
                    blk = Wb[b_, :, :, :, j]              # [kin2, d, ci]
                    w_np[par * C : par * C + C, 6 * b_ + 2 * s + i, :] = (
                        blk.transpose(2, 0, 1).reshape(C, 128)
                    )
        w_np[0:C, 6 * b_ + 4, :] = (
            Wb[b_, :, :, :, 8].transpose(2, 0, 1).reshape(C, 128)
        )
        w_np[C, 6 * b_ + 4, :] = b2q.reshape(NB, 128)[b_]

    mask_np = np.zeros((128, 8, K), np.float32)
    r = np.arange(128)
    for p in range(4):
        for i in range(2):
            mask_np[r, 2 * p + i, 4 * p + 2 * i + r // C] = 1.0

    const = C * math.log(2.0 * math.pi) + logdet
    bias_np = (-0.5 * const).astype(np.float32).reshape(K, 1)
    return w_np.astype(_FP8_NP), mask_np.astype(_FP8_NP), bias_np


def _make_in_maps(x, w_np, mask_np, bias_np, tloc=TLOC, ncores=NCORES):
    xq = np.asarray(x, np.float32)[0].astype(_FP8_NP).astype(np.float32)
    xpad = np.pad(xq, ((0, 0), (AR, TB + 2)))       # [C, AR+T+TB+2]
    in_maps = []
    for i in range(ncores):
        lo = xpad[:, tloc * i : tloc * i + tloc + AR + 1]
        hi = xpad[:, tloc * i + 1 : tloc * i + tloc + AR + 2]
        # xed col u = x_glob[core_start + u]; row 64 ones; rows 65-127 zeros;
        # cols [tloc, tloc+TB) zero dead-zone for the s2 second k-tile
        xed_np = np.zeros((128, tloc + TB), np.float32)
        xed_np[0:C, 0:tloc] = xpad[:, tloc * i + AR : tloc * i + AR + tloc]
        xed_np[C, :] = 1.0
        xed_np[:, tloc:] = 0.0
        in_maps.append(
            {
                "xin": np.ascontiguousarray(
                    np.concatenate([lo, hi], axis=0).astype(_FP8_NP)
                ),
                "xed": xed_np.astype(_FP8_NP),
                "wts": w_np,
                "maskd": mask_np,
                "biasd": bias_np,
            }
        )
    return in_maps


def _run(x, W, b, Sigma, trace=False):
    if "nc" not in _CACHE:
        _CACHE["nc"] = _build_program()
    nc = _CACHE["nc"]
    w_np, mask_np, bias_np = _prep_host(
        np.asarray(W, np.float32), np.asarray(b, np.float32),
        np.asarray(Sigma, np.float32),
    )
    in_maps = _make_in_maps(np.asarray(x, np.float32), w_np, mask_np, bias_np)
    res = run_bass_kernel_spmd(
        nc, in_maps, core_ids=list(range(NCORES)), trace=trace
    )
    outs = [res.results[i]["out"] for i in range(NCORES)]
    full = np.concatenate(outs, axis=1)[None]   # [1, K, T]
    return full.astype(np.float32), res


def kernel(x, W, b, Sigma):
    out, _ = _run(x, W, b, Sigma, trace=bool(int(os.environ.get("BASS_TRACE", "0"))))
    return out
